# revision 1
# baseline (speedup 1.0000x reference)
"""CausalSelfAttention TRN2 kernel: LN + QKV + causal attention + out_proj.

Sharding: 8 cores = 4 batches x 2 head-groups (8 heads each). Each core
computes its batch's LayerNorm, QKV for its heads, causal softmax attention,
and a partial out-projection over its heads' channels; the host sums the two
partials per batch.

Per-core layouts (SBUF partition dim first):
  hT   [c, t]   LN(x) transposed via PE, bf16
  qT/kT [o, t]  o = head*64+d; head pair (2i,2i+1) shares a 128-partition tile
  v    [t, (h, 65)] bf16, col 64 = ones (PV emits softmax sums as row 64)
  scores sT [tk, tq] per 128x512 tile, K=64 head-pairs row-tiled concurrently;
  exp on ACT (scale=1/8 fused) over valid columns only (tq_loc >= r for the
  diagonal tile at offset r); causality via a single 128-wide multiplicative
  [i>j] mask on DVE; PV: lhsT=v_ext, rhs=p[:, r:] -> out2t [65, tq]
  normalization: sums -> DRAM roundtrip -> 64-partition broadcast -> DVE mul
  out_proj: lhsT = A.T [j, t] f32r, rhs = woT [j, o] f32r
"""
import math
import sys

sys.path.insert(0, "/opt/trn_rl_repo")
sys.path.insert(0, "/opt/trn_rl_repo/concourse")

import numpy as np
import ml_dtypes

import concourse.bass as bass
import concourse.bacc as bacc
import concourse.mybir as mybir
import concourse.tile as tile
from concourse.bass_utils import run_bass_kernel_spmd

T, C, NH, DH = 2048, 1024, 16, 64
HC = 8            # heads per core
NT = T // 128     # 16 t-tiles
KC = C // 128     # 8 contraction tiles
W = 512           # tq block width
NJ = T // W       # 4 q blocks
NP = HC // 2      # 4 head pairs
GS = 2            # kt tiles per scores/exp group
F32, F32R, BF16 = mybir.dt.float32, mybir.dt.float32r, mybir.dt.bfloat16
AF = mybir.ActivationFunctionType

_CACHE = {}


def _build(beta_nonzero):
    nc = bacc.Bacc("TRN2", target_bir_lowering=False, debug=False)
    dx = nc.dram_tensor("x", [T, C], F32, kind="ExternalInput")
    dwq = nc.dram_tensor("wq", [KC, 128, 512], BF16, kind="ExternalInput")
    dwk = nc.dram_tensor("wk", [KC, 128, 512], BF16, kind="ExternalInput")
    dwv = nc.dram_tensor("wv", [KC, 128, 512], BF16, kind="ExternalInput")
    dwo = nc.dram_tensor("wo", [NP, 128, 1024], F32R, kind="ExternalInput")
    dmask = nc.dram_tensor("masks", [4, 128, 512], BF16, kind="ExternalInput")
    did = nc.dram_tensor("ident", [128, 128], BF16, kind="ExternalInput")
    dbeta = nc.dram_tensor("betab", [1, C], F32, kind="ExternalInput")
    dout = nc.dram_tensor("out", [T, C], F32, kind="ExternalOutput")

    with tile.TileContext(nc) as tc:
        cst = tc.alloc_tile_pool(name="cst", bufs=1)
        ident = cst.tile([128, 128], BF16)
        mask_sb = cst.tile([128, 4, 512], BF16)
        wo_sb = cst.tile([128, NP, 1024], F32R)
        eps = cst.tile([128, 1], F32)
        nc.sync.dma_start(ident[:], did[:])
        nc.vector.memset(eps[:], 1e-5)
        att = tc.alloc_tile_pool(name="att", bufs=1)
        qT = att.tile([128, NP, T], BF16)
        kT = att.tile([128, NP, T], BF16)
        v_sb = att.tile([128, NT, HC, 65], BF16)
        nc.vector.memset(v_sb[:, :, :, 64:65], 1.0)

        # ---------------- Phase A: LN -> transpose -> QKV ----------------
        with tc.tile_pool(name="wqkv", bufs=1) as wp, \
             tc.tile_pool(name="xp", bufs=3) as xp, \
             tc.tile_pool(name="hp", bufs=3) as hp, \
             tc.tile_pool(name="hT", bufs=1) as hTp, \
             tc.tile_pool(name="st", bufs=4) as stp, \
             tc.tile_pool(name="tps", bufs=3, space="PSUM") as tps, \
             tc.tile_pool(name="qkps", bufs=4, space="PSUM") as qkps:
            wq_sb = wp.tile([128, KC, 512], BF16, tag="w")
            wk_sb = wp.tile([128, KC, 512], BF16, tag="w2")
            wv_sb = wp.tile([128, KC, 512], BF16, tag="w3")
            if beta_nonzero:
                beta_sb = wp.tile([128, C], F32, tag="beta")
                bap = dbeta[0:1, :]
                nc.gpsimd.dma_start(
                    out=beta_sb[:],
                    in_=bass.AP(tensor=bap.tensor, offset=bap.offset,
                                ap=[[0, 128], bap.ap[1]]))
            hT = hTp.tile([128, KC, T], BF16)
            for tb in range(NJ):
                for tt in range(4 * tb, 4 * tb + 4):
                    xt = xp.tile([128, C], F32)
                    nc.sync.dma_start(xt[:], dx[tt * 128:(tt + 1) * 128, :])
                    if tb == 0 and tt < 2:
                        for kc in range(4 * tt, 4 * tt + 4):
                            nc.sync.dma_start(wq_sb[:, kc, :], dwq[kc])
                            nc.sync.dma_start(wk_sb[:, kc, :], dwk[kc])
                            nc.sync.dma_start(wv_sb[:, kc, :], dwv[kc])
                    stats = stp.tile([128, 2, 6], F32, tag="stats")
                    xg = xt[:].rearrange("p (g d) -> p g d", g=2)
                    for g in range(2):
                        nc.vector.bn_stats(stats[:, g, :], xg[:, g, :])
                    mv = stp.tile([128, 2], F32, tag="mv")
                    nc.vector.bn_aggr(mv[:], stats[:])
                    sd = stp.tile([128, 1], F32, tag="sd")
                    nc.scalar.activation(sd[:], mv[:, 1:2], AF.Sqrt, bias=eps[:], scale=1.0)
                    nc.vector.reciprocal(sd[:], sd[:])
                    ht = hp.tile([128, C], BF16)
                    nc.vector.tensor_scalar(
                        out=ht[:], in0=xt[:], scalar1=mv[:, 0:1], scalar2=sd[:],
                        op0=mybir.AluOpType.subtract, op1=mybir.AluOpType.mult)
                    if beta_nonzero:
                        nc.vector.tensor_add(ht[:], ht[:], beta_sb[:])
                    tp = tps.tile([128, KC, 128], BF16)
                    for kc in range(KC):
                        nc.tensor.transpose(tp[:, kc, :], ht[:, kc * 128:(kc + 1) * 128], ident[:])
                    nc.vector.tensor_copy(hT[:, :, tt * 128:(tt + 1) * 128], tp[:])
                for ot in range(NP):
                    pq = qkps.tile([128, 512], F32, tag="ps")
                    for kc in range(KC):
                        nc.tensor.matmul(pq[:], wq_sb[:, kc, ot * 128:(ot + 1) * 128],
                                         hT[:, kc, tb * 512:(tb + 1) * 512],
                                         start=(kc == 0), stop=(kc == KC - 1))
                    nc.vector.tensor_copy(qT[:, ot, tb * 512:(tb + 1) * 512], pq[:])
                    pk = qkps.tile([128, 512], F32, tag="ps")
                    for kc in range(KC):
                        nc.tensor.matmul(pk[:], wk_sb[:, kc, ot * 128:(ot + 1) * 128],
                                         hT[:, kc, tb * 512:(tb + 1) * 512],
                                         start=(kc == 0), stop=(kc == KC - 1))
                    nc.vector.tensor_copy(kT[:, ot, tb * 512:(tb + 1) * 512], pk[:])
                for tt in range(4 * tb, 4 * tb + 4):
                    pv = qkps.tile([128, 512], F32, tag="ps")
                    for kc in range(KC):
                        nc.tensor.matmul(pv[:], hT[:, kc, tt * 128:(tt + 1) * 128],
                                         wv_sb[:, kc, :],
                                         start=(kc == 0), stop=(kc == KC - 1))
                    nc.vector.tensor_copy(
                        v_sb[:, tt, :, 0:64],
                        pv[:].rearrange("p (h d) -> p h d", h=HC))

        # ---------------- Phase B: attention + out_proj ----------------
        with tc.tile_pool(name="sps", bufs=3, space="PSUM") as sps, \
             tc.tile_pool(name="ops", bufs=2, space="PSUM") as ops, \
             tc.tile_pool(name="pp", bufs=6) as ppool, \
             tc.tile_pool(name="up", bufs=10) as upool, \
             tc.tile_pool(name="facp", bufs=8) as facp, \
             tc.tile_pool(name="atmp", bufs=4) as atmp, \
             tc.tile_pool(name="sums", bufs=1) as sums, \
             tc.tile_pool(name="atp", bufs=1) as atp, \
             tc.tile_pool(name="outp", bufs=4) as outp, \
             tc.tile_pool(name="drp", bufs=1, space="DRAM") as drp:
            for r in range(4):
                nc.sync.dma_start(mask_sb[:, r, :], dmask[r])
            for jp in range(NP):
                nc.sync.dma_start(wo_sb[:, jp, :], dwo[jp])
            s8 = sums.tile([8, NJ, 512], F32)
            recip8 = sums.tile([8, NJ, 512], F32)
            AT = atp.tile([128, NP, NJ, 512], F32R)
            drec = drp.tile([8, NJ, 512], F32)

            def emit_attention(J):
                nkt = 4 * J + 4
                u_tiles = []
                for hpair in range(NP):
                    hA, hB = 2 * hpair, 2 * hpair + 1
                    poA = ops.tile([65, 512], F32, tag="po")
                    poB = ops.tile([65, 512], F32, tag="po")
                    for g in range(nkt // GS):
                        kts = list(range(g * GS, (g + 1) * GS))
                        spA = sps.tile([128, GS, 512], F32, tag="sp")
                        spB = sps.tile([128, GS, 512], F32, tag="sp")
                        ptA = ppool.tile([128, GS, 512], BF16, tag="pt")
                        ptB = ppool.tile([128, GS, 512], BF16, tag="pt")
                        # column offset r: tq_loc < r is fully masked for
                        # diagonal tile kt (r = 128*(kt-4J)); skip those columns
                        offs = [max(0, (kt - 4 * J) * 128) for kt in kts]
                        for i, kt in enumerate(kts):
                            for sp, base in ((spA, 0), (spB, 64)):
                                nc.tensor.matmul(
                                    sp[:, i, :],
                                    kT[base:base + 64, hpair, kt * 128:(kt + 1) * 128],
                                    qT[base:base + 64, hpair, J * 512:(J + 1) * 512],
                                    start=True, stop=True,
                                    tile_position=(base, 0))
                        if offs == [0] * GS:
                            for sp, pt in ((spA, ptA), (spB, ptB)):
                                nc.scalar.activation(
                                    pt[:].rearrange("p g f -> p (g f)"),
                                    sp[:].rearrange("p g f -> p (g f)"),
                                    AF.Exp, scale=0.125)
                        else:
                            for i, kt in enumerate(kts):
                                for sp, pt in ((spA, ptA), (spB, ptB)):
                                    nc.scalar.activation(
                                        pt[:, i, offs[i]:512],
                                        sp[:, i, offs[i]:512],
                                        AF.Exp, scale=0.125)
                        for i, kt in enumerate(kts):
                            if kt - 4 * J >= 0:
                                r = offs[i]
                                for pt in (ptA, ptB):
                                    nc.vector.tensor_mul(pt[:, i, r:r + 128],
                                                         pt[:, i, r:r + 128],
                                                         mask_sb[:, 0, 0:128])
                        for i, kt in enumerate(kts):
                            r = offs[i]
                            for po, h, pt in ((poA, hA, ptA), (poB, hB, ptB)):
                                nc.tensor.matmul(
                                    po[:, r:512], v_sb[:, kt, h, :], pt[:, i, r:512],
                                    start=(kt == 0), stop=(kt == nkt - 1))
                    uA = upool.tile([65, 512], F32, tag="u")
                    uB = upool.tile([65, 512], F32, tag="u")
                    nc.vector.tensor_copy(uA[:], poA[:])
                    nc.vector.tensor_copy(uB[:], poB[:])
                    u_tiles.append((uA, uB))
                    nc.sync.dma_start(s8[hA:hA + 1, J, :], uA[64:65, :])
                    nc.sync.dma_start(s8[hB:hB + 1, J, :], uB[64:65, :])
                nc.vector.reciprocal(recip8[:, J, :], s8[:, J, :])
                nc.sync.dma_start(drec[:, J, :], recip8[:, J, :])
                for hpair in range(NP):
                    uA, uB = u_tiles[hpair]
                    for hh, h, u in ((0, 2 * hpair, uA), (1, 2 * hpair + 1, uB)):
                        fac = facp.tile([64, 512], F32)
                        row = drec[h:h + 1, J, :]
                        nc.sync.dma_start(
                            fac[:],
                            bass.AP(tensor=row.tensor, offset=row.offset,
                                    ap=[[0, 64], row.ap[-1]]))
                        if hh == 0:
                            nc.vector.tensor_mul(AT[0:64, hpair, J, :],
                                                 u[0:64, :], fac[:])
                        else:
                            at = atmp.tile([64, 512], F32R)
                            nc.vector.tensor_mul(at[:], u[0:64, :], fac[:])
                            nc.sync.dma_start(AT[64:128, hpair, J, :], at[:])

            def emit_out_proj(J):
                for tc4 in range(4):
                    for ob in range(2):
                        pp_ = sps.tile([128, 512], F32, tag="sp")
                        for hpair in range(NP):
                            nc.tensor.matmul(
                                pp_[:], AT[:, hpair, J, tc4 * 128:(tc4 + 1) * 128],
                                wo_sb[:, hpair, ob * 512:(ob + 1) * 512],
                                start=(hpair == 0), stop=(hpair == NP - 1))
                        ot_ = outp.tile([128, 512], F32)
                        nc.vector.tensor_copy(ot_[:], pp_[:])
                        t0 = J * 512 + tc4 * 128
                        nc.sync.dma_start(dout[t0:t0 + 128, ob * 512:(ob + 1) * 512],
                                          ot_[:])

            for J in range(NJ):
                emit_attention(J)
                if J > 0:
                    emit_out_proj(J - 1)
            emit_out_proj(NJ - 1)
        att.release()
        cst.release()
    nc.compile()
    return nc


def kernel(x, gamma, beta, w_qkv, w_out):
    x = np.asarray(x, dtype=np.float32)
    gamma = np.asarray(gamma, dtype=np.float32)
    beta = np.asarray(beta, dtype=np.float32)
    w_qkv = np.asarray(w_qkv, dtype=np.float32)
    w_out = np.asarray(w_out, dtype=np.float32)
    B = x.shape[0]
    beta_nonzero = bool(np.any(beta != 0.0))
    key = ("k", beta_nonzero)
    if key not in _CACHE:
        _CACHE[key] = _build(beta_nonzero)
    nc = _CACHE[key]

    i128, j128 = np.indices((128, 512))
    masks = np.stack([np.where(i128 + r > j128, 0.0, 1.0)
                      for r in (0, 128, 256, 384)]).astype(ml_dtypes.bfloat16)
    ident = np.eye(128, dtype=ml_dtypes.bfloat16)
    betab = beta.reshape(1, C)

    in_maps = []
    for core in range(8):
        b, g = core // 2, core % 2
        sl = slice(g * 512, (g + 1) * 512)
        wq = (w_qkv[0 * C:1 * C][sl] * gamma[None, :]).T.copy()      # [1024, 512]
        wk = (w_qkv[1 * C:2 * C][sl] * gamma[None, :]).T.copy()
        wv = (w_qkv[2 * C:3 * C][sl] * gamma[None, :]).T.copy()
        wo = w_out[:, sl].T.copy()                                    # [512, 1024]
        in_maps.append({
            "x": np.ascontiguousarray(x[b]),
            "wq": wq.reshape(KC, 128, 512).astype(ml_dtypes.bfloat16),
            "wk": wk.reshape(KC, 128, 512).astype(ml_dtypes.bfloat16),
            "wv": wv.reshape(KC, 128, 512).astype(ml_dtypes.bfloat16),
            "wo": np.ascontiguousarray(wo.reshape(NP, 128, 1024)),
            "masks": masks,
            "ident": ident,
            "betab": betab,
        })
    res = run_bass_kernel_spmd(nc, in_maps, core_ids=list(range(8)))
    out = np.empty((B, T, C), dtype=np.float32)
    for b in range(B):
        out[b] = res.results[2 * b]["out"] + res.results[2 * b + 1]["out"]
    return out

